# revision 29
# baseline (speedup 1.0000x reference)
"""AttentionMV Trainium2 kernel.

Computes, for each batch row b:
    ht     = tanh(enc[b] @ W + b_bias)          # (T, E)
    scores = ht @ ctx[b]                        # (T,)
    at     = softmax(scores)
    out[b] = at @ ht                            # (E,)

Sharding: data-parallel over batch across 8 NeuronCores (4 rows each);
W / b replicated. No cross-core communication.

Implementation notes:
  - The big matmul runs in float16 (10 mantissa bits): e2e l2 rel err
    ~2.7e-3 (vs gate 2e-2; bf16 would be ~2e-2, f32r ~8e-4 but slower).
    fp16 streams at 1 col/cycle like bf16, and bacc splits 2-byte matmuls
    into LDWEIGHTS+MATMUL pairs. Measured HW floor is ~259 ns per
    [128x128]x[128x512] MM under 8-core load (PE P0-throttled to ~2.0
    GHz; ~216 ns when the machine is quiet) -- dtype/order-independent,
    so the 1024-MM stream per core is ~221-265 us and is the roofline.
  - fp16 also halves enc DMA (16.8 MB/core) and SBUF footprint; DMA is
    fully hidden (measured: removing 7/8 of enc DMA changes nothing).
  - enc is pre-transposed on the host to (E, T) per batch so the PE
    contraction tiles (E on partitions) load with contiguous free dims.
  - ht/ctx/acc/scratch are fp16 (DVE 16-bit 2x mode); DVE scalar operands
    (exps columns) must be f32, so exps stays f32r with bitcasts.
  - Matmul order is k-outer (PSUM banks psA/psB alternate per MM).
  - Pooling of batch i runs on DVE (serial scalar_tensor_tensor chain)
    interleaved into batch i+1's matmul stream; the final ones x acc
    partition-reduce on PE is deferred REDUCE_DELAY chains so the PE
    stream never waits on the DVE chain. The last batch splits pooling
    PE/DVE half-half (combined in one PSUM accumulation group) to halve
    the exposed tail.
  - Softmax uses DVE free-dim reduce + GPSIMD partition_all_reduce; the
    1/Z normalization is folded into the host-side divide by zout.
  - The dyn-loop bench variant uses branch-prefetch hints + staggered
    semaphore reset at the For_i back-edge (no all-engine barrier).
  - Steady-state HW time ~235 us/core quiet, ~290-330 us under heavy
    machine load (PE throttle + HBM contention from co-tenants).
"""
import contextlib

import numpy as np
import ml_dtypes

import concourse.bacc as bacc
import concourse.bass_isa as bass_isa
import concourse.mybir as mybir
from concourse.bass_utils import run_bass_kernel_spmd
from concourse.tile import TileContext, add_dep_helper

B, T, E = 32, 2048, 1024
NCORES = 8
BPC = B // NCORES          # batches per core
NT = T // 128              # 16 t-tiles per batch
NK = E // 128              # 8 k-tiles (contraction)
NT512 = T // 512           # 4 groups of 4 t-tiles
POOL_DELAY = 2             # m-chains of next batch before prev pooling
REDUCE_DELAY = 12          # m-chains before prev batch's PE pool-reduce

f32 = mybir.dt.float32
f32r = mybir.dt.float32r
f16 = mybir.dt.float16
bf16 = mybir.dt.bfloat16
AF = mybir.ActivationFunctionType
ALU = mybir.AluOpType
AX = mybir.AxisListType


def _build(with_bias, repeat=1, dyn_loop=False, ablate=""):
    ab = set(ablate.split(",")) if ablate else set()
    pool_delay = POOL_DELAY
    psum_bufs, et_bufs = 3, 2
    for tok in list(ab):
        if tok.startswith("pd"):
            pool_delay = int(tok[2:]); ab.discard(tok)
        elif tok.startswith("psum"):
            psum_bufs = int(tok[4:]); ab.discard(tok)
        elif tok.startswith("et"):
            et_bufs = int(tok[2:]); ab.discard(tok)
    dve_pool = "nodvepool" not in ab
    ab.discard("dvepool"); ab.discard("nodvepool")
    psplit = "nopsplit" not in ab
    ab.discard("psplit"); ab.discard("nopsplit")
    # k-outer (PSUM-bank-alternating) matmul order is the default: psA/psB
    # drains overlap the other bank's fills. "nouter" reverts to n-outer.
    kouter = "nouter" not in ab
    ab.discard("kouter"); ab.discard("nouter")
    # dyn-loop back-edge: branch-prefetch hints (body >> one IRAM block) and
    # staggered semaphore reset (no all-engine barrier) are on by default
    loop_hint = "nohint" not in ab
    ab.discard("hint"); ab.discard("nohint")
    loop_stag = "nostag" not in ab
    ab.discard("stag"); ab.discard("nostag")
    nc = bacc.Bacc(None)
    enc = nc.declare_dram_parameter("enc", [BPC, E, T], f16, isOutput=False)
    if dyn_loop:
        nrep = nc.declare_dram_parameter("nrep", [1, 1], mybir.dt.int32,
                                         isOutput=False)
    ctxv = nc.declare_dram_parameter("ctx", [BPC, E], f16, isOutput=False)
    W = nc.declare_dram_parameter("W", [E, E], f16, isOutput=False)
    bvec = nc.declare_dram_parameter("b", [2, E], f32, isOutput=False)
    out = nc.declare_dram_parameter("out", [BPC, E], f32, isOutput=True)
    zout = nc.declare_dram_parameter("zout", [BPC, 128], f32, isOutput=True)

    with TileContext(nc) as tc:
        with (
            tc.tile_pool(name="const", bufs=1) as cpool,
            tc.tile_pool(name="ht2", bufs=2) as htpool2,
            tc.tile_pool(name="ht1", bufs=1) as htpool1,
            tc.tile_pool(name="et", bufs=et_bufs) as etpool,
            tc.tile_pool(name="work", bufs=2) as wpool,
            tc.tile_pool(name="psum", bufs=psum_bufs, space="PSUM") as psum_pool,
            tc.tile_pool(name="ppool", bufs=1, space="PSUM") as ppool,
        ):
            # --- constants ---
            # In the single-shot program, W tile loads are interleaved with
            # the first batch's enc tile loads so the first matmul chain
            # starts after ~0.5MB of DMA instead of ~6MB.
            w_t = []
            for k in range(NK):
                wt = cpool.tile([128, E], f16, tag=f"w{k}", name=f"w_t{k}")
                if dyn_loop:
                    nc.sync.dma_start(out=wt[:], in_=W[k * 128:(k + 1) * 128, :])
                w_t.append(wt)
            w_loaded = dyn_loop
            if with_bias:
                b_f = cpool.tile([2, E], f32)
                nc.sync.dma_start(out=b_f[:], in_=bvec[:])
                b_t = cpool.tile([2, E], bf16)
                nc.vector.tensor_copy(b_t[:], b_f[:])
                zero_s = cpool.tile([2, 128], f32)
                nc.vector.memset(zero_s[:], 0.0)
                ones_b = cpool.tile([2, 128], bf16)
                nc.scalar.activation(ones_b[:], zero_s[:], AF.Copy,
                                     bias=1.0, scale=0.0)

            if dve_pool:
                zero_o = cpool.tile([128, 1], f32)
                nc.vector.memset(zero_o[:], 0.0)
                ones_r = cpool.tile([128, 1], f16)
                nc.scalar.activation(ones_r[:], zero_o[:], AF.Copy,
                                     bias=1.0, scale=0.0)

            # per-batch state carried between emission phases
            state = {}

            def pe(bi):
                return bi

            def mm_pair(dst, lhsT, rhs, start, stop):
                return nc.tensor.matmul(dst, lhsT, rhs, start=start,
                                        stop=stop)
            loop_cm = contextlib.nullcontext()
            if dyn_loop:
                nrep_t = cpool.tile([1, 1], mybir.dt.int32)
                nc.sync.dma_start(out=nrep_t[:], in_=nrep[:])
                nval = nc.values_load(nrep_t[0:1, 0:1])
                hints = (tuple(mybir.EngineType) if loop_hint else ())
                loop_cm = tc.For_i(0, nval, 1, hint_engines=hints,
                                   staggered_reset=loop_stag)

            pending_reduce = {}

            def _emit_acc_chains(i, exps, ht_b, t0, t1):
                # sum_{t in [t0,t1)} ht[t] * exps[:, t] via two parity
                # chains, each ping-ponging between two tiles; returns the
                # combined [128, E] f16 tile.
                acc = [[wpool.tile([128, E], f16, tag=f"acc{j}{s}",
                                   name=f"acc_{i}_{j}{s}") for s in range(2)]
                       for j in range(2)]
                steps = {0: 0, 1: 0}
                lastt = {}
                ts = list(range(t0, t1))
                for idx, t in enumerate(ts):
                    j = idx % 2
                    s = steps[j]
                    if s == 0:
                        nc.vector.tensor_scalar_mul(
                            acc[j][0][:], ht_b[t][:],
                            exps[:, t:t + 1].bitcast(f32))
                    else:
                        nc.vector.scalar_tensor_tensor(
                            out=acc[j][s % 2][:],
                            in0=ht_b[t][:],
                            scalar=exps[:, t:t + 1].bitcast(f32),
                            in1=acc[j][(s + 1) % 2][:],
                            op0=ALU.mult, op1=ALU.add)
                    lastt[j] = s % 2
                    steps[j] += 1
                if steps[1] == 0:
                    return acc[0][lastt[0]]
                comb = wpool.tile([128, E], f16, tag="acccomb",
                                  name=f"acccomb_{i}")
                nc.vector.tensor_add(comb[:], acc[0][lastt[0]][:],
                                     acc[1][lastt[1]][:])
                return comb

            def emit_pooling(i):
                if "pool" in ab:
                    return
                exps, ht_b, rz, b = state[i]
                # last batch: PE pooling (PE is idle at the tail and its
                # 32-matmul chain is ~10us shorter than the serial DVE chain)
                last_i = repeat * BPC - 1
                if dve_pool and i != last_i:
                    # acc = sum_t ht[t] * exps[:, t] on DVE as two
                    # independent even/odd-parity chains (interleaved on the
                    # DVE FIFO, so dependent-op drain bubbles are hidden and
                    # the serial depth halves); the final PE partition-reduce
                    # is deferred to emit_pool_reduce so the PE stream never
                    # waits on the chains.
                    last = _emit_acc_chains(i, exps, ht_b, 0, NT)
                    pending_reduce[i] = (last, b)
                else:
                    # tail batch: split pooling across PE (t < SPLIT, direct
                    # exps16 x ht matmuls) and DVE (t >= SPLIT, serial acc
                    # chain) so the exposed tail is ~halved; the DVE partial
                    # joins the same PSUM accumulation group via a final
                    # ones x acc matmul with start=False.
                    SPLIT = NT // 2
                    exps16 = wpool.tile([128, NT], f16, tag="exps16",
                                        name=f"exps16_{i}")
                    nc.vector.tensor_copy(exps16[:], exps[:].bitcast(f32))
                    last = _emit_acc_chains(i, exps, ht_b, SPLIT, NT)
                    ps_o = ppool.tile([1, E], f32, tag="ps_o", name=f"ps_o{i}")
                    for n in range(2):
                        sl = slice(n * 512, (n + 1) * 512)
                        for t in range(SPLIT):
                            pe(nc.tensor.matmul(ps_o[:, sl],
                                                exps16[:, t:t + 1],
                                                ht_b[t][:, sl],
                                                start=(t == 0),
                                                stop=False))
                    for n in range(2):
                        sl = slice(n * 512, (n + 1) * 512)
                        pe(nc.tensor.matmul(ps_o[:, sl], ones_r[:],
                                            last[:, sl],
                                            start=False, stop=(n == 1)))
                    _emit_out(i, ps_o, state[i][3])

            def emit_pool_reduce(i):
                if i not in pending_reduce:
                    return
                last, b = pending_reduce.pop(i)
                ps_o = ppool.tile([1, E], f32, tag="ps_o", name=f"ps_o{i}")
                for n in range(2):
                    sl = slice(n * 512, (n + 1) * 512)
                    pe(nc.tensor.matmul(ps_o[:, sl], ones_r[:], last[:, sl],
                                        start=True, stop=True))
                _emit_out(i, ps_o, b)

            def _emit_out(i, ps_o, b):
                out_sb = wpool.tile([1, E], f32, tag="out_sb", name=f"out_sb{i}")
                nc.scalar.activation(out_sb[:], ps_o[:], AF.Copy)
                nc.sync.dma_start(out=out[b:b + 1, :], in_=out_sb[:])

            with loop_cm:
                for i in range(repeat * BPC):
                    b = i % BPC
                    ctx_b = wpool.tile([128, E], f16, tag="ctx_b", name=f"ctx_b{i}")
                    nc.sync.dma_start(out=ctx_b[:],
                                      in_=ctxv[b:b + 1, :].to_broadcast((128, E)))
                    scores = wpool.tile([128, NT], f32, tag="scores",
                                        name=f"scores{i}")
                    # tiles written before prev batch's pooling is emitted need
                    # double buffering; later ones can reuse a single slot
                    ht = [(htpool2 if t < pool_delay + 2 else htpool1).tile(
                              [128, E], f16, tag=f"ht{t}", name=f"ht_{i}_{t}")
                          for t in range(NT)]

                    chain_idx = 0
                    et_tiles = None
                    for t512 in range(NT512):
                        first_group = not w_loaded and psplit and not ab
                        et_tiles = []
                        for k in range(NK):
                            if "dma" in ab and k > 0:
                                et_tiles.append(et_tiles[0])
                                continue
                            et = etpool.tile([128, 512], f16, tag=f"et{k}",
                                             name=f"et_{i}_{t512}_{k}")
                            src = enc[b, k * 128:(k + 1) * 128,
                                      t512 * 512:(t512 + 1) * 512]
                            if not w_loaded:
                                if first_group:
                                    # n=0 half of W first: the first 4 chains
                                    # only need cols 0:512, so the first
                                    # matmuls start after ~4MB of DMA not 6MB
                                    nc.sync.dma_start(
                                        out=w_t[k][:, 0:512],
                                        in_=W[k * 128:(k + 1) * 128, 0:512])
                                else:
                                    nc.sync.dma_start(
                                        out=w_t[k][:],
                                        in_=W[k * 128:(k + 1) * 128, :])
                            if "dma" not in ab or k == 0:
                                nc.sync.dma_start(out=et[:], in_=src)
                            et_tiles.append(et)
                        if first_group:
                            for k in range(NK):
                                nc.sync.dma_start(
                                    out=w_t[k][:, 512:1024],
                                    in_=W[k * 128:(k + 1) * 128, 512:1024])
                        w_loaded = True
                        if first_group:
                            # n-outer over the whole group: all four m-chains
                            # run on the n=0 W halves before any n=1 chain
                            ps_h = {}
                            for nn in range(2):
                                nsl = slice(nn * 512, (nn + 1) * 512)
                                for m in range(4):
                                    t = t512 * 4 + m
                                    msl = slice(m * 128, (m + 1) * 128)
                                    tag = "psA" if nn == 0 else "psB"
                                    ph = psum_pool.tile(
                                        [128, 512], f32, tag=tag,
                                        name=f"ps{tag[-1]}_{i}_{t}")
                                    ps_h[(m, nn)] = ph
                                    for k in range(NK):
                                        mm_pair(
                                            ph[:], et_tiles[k][:, msl],
                                            w_t[k][:, nsl], start=(k == 0),
                                            stop=(k == NK - 1
                                                  and not with_bias))
                                    if with_bias:
                                        pe(nc.tensor.matmul(
                                            ph[:], ones_b[:], b_t[:, nsl],
                                            start=False, stop=True))
                                    nc.scalar.activation(ht[t][:, nsl],
                                                         ph[:], AF.Tanh)
                                    if nn == 1:
                                        scratch = wpool.tile(
                                            [128, E], f16, tag="scratch",
                                            name=f"scr_{i}_{t}")
                                        nc.vector.scalar_tensor_tensor(
                                            out=scratch[:],
                                            in0=ht[t][:],
                                            scalar=1.0, in1=ctx_b[:],
                                            op0=ALU.mult, op1=ALU.mult,
                                            accum_out=scores[:, t:t + 1])
                                        chain_idx += 1
                            continue
                        for m in range(4):
                            t = t512 * 4 + m
                            msl = slice(m * 128, (m + 1) * 128)
                            if psplit:
                                psA = psum_pool.tile([128, 512], f32, tag="psA",
                                                     name=f"psA_{i}_{t}")
                                psB = psum_pool.tile([128, 512], f32, tag="psB",
                                                     name=f"psB_{i}_{t}")
                                ps_halves = [psA, psB]
                            else:
                                ps = psum_pool.tile([128, E], f32, tag="ps",
                                                    name=f"ps_{i}_{t}")
                            nk_eff = 1 if "mm" in ab else NK
                            korder = kouter
                            if korder:
                                seq = [(k, n) for k in range(nk_eff)
                                       for n in range(2)]
                            else:
                                seq = [(k, n) for n in range(2)
                                       for k in range(nk_eff)]
                            for k, n in seq:
                                nsl = slice(n * 512, (n + 1) * 512)
                                dst = (ps_halves[n][:] if psplit
                                       else ps[:, nsl])
                                mm_pair(
                                    dst, et_tiles[k][:, msl],
                                    w_t[k][:, nsl], start=(k == 0),
                                    stop=(k == nk_eff - 1 and not with_bias))
                            if with_bias:
                                for n in range(2):
                                    nsl = slice(n * 512, (n + 1) * 512)
                                    dst = (ps_halves[n][:] if psplit
                                           else ps[:, nsl])
                                    pe(nc.tensor.matmul(dst, ones_b[:],
                                                        b_t[:, nsl],
                                                        start=False,
                                                        stop=True))
                            if psplit:
                                nc.scalar.activation(ht[t][:, 0:512],
                                                     psA[:], AF.Tanh)
                                nc.scalar.activation(ht[t][:, 512:1024],
                                                     psB[:], AF.Tanh)
                            else:
                                nc.scalar.activation(ht[t][:], ps[:], AF.Tanh)
                            scratch = wpool.tile([128, E], f16, tag="scratch",
                                                 name=f"scr_{i}_{t}")
                            if "stt" not in ab:
                                nc.vector.scalar_tensor_tensor(
                                    out=scratch[:], in0=ht[t][:],
                                    scalar=1.0, in1=ctx_b[:], op0=ALU.mult,
                                    op1=ALU.mult, accum_out=scores[:, t:t + 1])
                            elif t == 0:
                                nc.vector.memset(scores[:], 0.5)
                            chain_idx += 1
                            if i > 0 and chain_idx == pool_delay:
                                emit_pooling(i - 1)
                            if i > 0 and chain_idx == REDUCE_DELAY:
                                emit_pool_reduce(i - 1)

                    # softmax for batch b
                    if "softmax" in ab:
                        exps = wpool.tile([128, NT], f32r, tag="exps",
                                          name=f"exps{i}")
                        nc.vector.memset(exps[:].bitcast(f32), 0.5)
                        state[i] = (exps, ht, None, b)
                        continue
                    rmax = wpool.tile([128, 1], f32, tag="rmax", name=f"rmax{i}")
                    nc.vector.tensor_reduce(rmax[:], scores[:], axis=AX.X,
                                            op=ALU.max)
                    m128 = wpool.tile([128, 1], f32, tag="m128", name=f"m128{i}")
                    nc.gpsimd.partition_all_reduce(
                        m128[:], rmax[:], channels=128,
                        reduce_op=bass_isa.ReduceOp.max)
                    negm = wpool.tile([128, 1], f32, tag="negm", name=f"negm{i}")
                    nc.scalar.activation(negm[:], m128[:], AF.Copy, scale=-1.0)
                    exps = wpool.tile([128, NT], f32r, tag="exps", name=f"exps{i}")
                    zrow = wpool.tile([128, 1], f32, tag="zrow", name=f"zrow{i}")
                    nc.scalar.activation(exps[:], scores[:], AF.Exp, bias=negm[:],
                                         accum_out=zrow[:])
                    nc.sync.dma_start(out=zout[b:b + 1, :], in_=zrow[:])
                    state[i] = (exps, ht, None, b)

                emit_pooling(repeat * BPC - 1)
            state.clear()
    nc.finalize()
    return nc


_cache = {}


def _get_nc(with_bias, repeat=1, dyn_loop=False, ablate=""):
    key = (with_bias, repeat, dyn_loop, ablate)
    if key not in _cache:
        _cache[key] = _build(with_bias, repeat, dyn_loop, ablate)
    return _cache[key]


def _run(enc, ctx, W, b, trace=False, tmpdir=None):
    enc = np.asarray(enc, dtype=np.float32)
    ctx = np.ascontiguousarray(np.asarray(ctx, dtype=np.float32))
    W = np.ascontiguousarray(np.asarray(W, dtype=np.float32))
    b = np.asarray(b, dtype=np.float32).reshape(1, E)

    with_bias = bool(np.any(b))
    b_hi = b.astype(ml_dtypes.bfloat16).astype(np.float32)
    b_lo = (b - b_hi).astype(ml_dtypes.bfloat16).astype(np.float32)
    b2 = np.concatenate([b_hi, b_lo], axis=0)

    nc = _get_nc(with_bias)
    enc16 = enc.astype(np.float16)
    W16 = W.astype(np.float16)
    ctx16 = ctx.astype(np.float16)
    in_maps = [
        {"enc": np.ascontiguousarray(
             enc16[c * BPC:(c + 1) * BPC].transpose(0, 2, 1)),
         "ctx": ctx16[c * BPC:(c + 1) * BPC],
         "W": W16, "b": b2}
        for c in range(NCORES)
    ]
    res = run_bass_kernel_spmd(nc, in_maps, list(range(NCORES)),
                               trace=trace, tmpdir=tmpdir)
    outp = np.concatenate([res.results[c]["out"] for c in range(NCORES)],
                          axis=0).astype(np.float32)
    zsum = np.concatenate([res.results[c]["zout"] for c in range(NCORES)],
                          axis=0).astype(np.float64).sum(axis=1)
    outp = (outp / zsum[:, None]).astype(np.float32)
    return outp, res


def kernel(enc, ctx, W, b):
    outp, _ = _run(enc, ctx, W, b)
    return outp



# revision 33
# speedup vs baseline: 1.2490x; 1.2490x over previous
"""AttentionMV Trainium2 kernel.

Computes, for each batch row b:
    ht     = tanh(enc[b] @ W + b_bias)          # (T, E)
    scores = ht @ ctx[b]                        # (T,)
    at     = softmax(scores)
    out[b] = at @ ht                            # (E,)

Sharding: data-parallel over batch across 8 NeuronCores (4 rows each);
W / b replicated. No cross-core communication.

Implementation notes:
  - The big matmul runs in float16 (10 mantissa bits): e2e l2 rel err
    ~2.7e-3 (vs gate 2e-2; bf16 would be ~2e-2, f32r ~8e-4 but slower).
    fp16 streams at 1 col/cycle like bf16, and bacc splits 2-byte matmuls
    into LDWEIGHTS+MATMUL pairs. Measured HW floor is ~259 ns per
    [128x128]x[128x512] MM under 8-core load (PE P0-throttled to ~2.0
    GHz; ~216 ns when the machine is quiet) -- dtype/order-independent,
    so the 1024-MM stream per core is ~221-265 us and is the roofline.
  - fp16 also halves enc DMA (16.8 MB/core) and SBUF footprint; DMA is
    fully hidden (measured: removing 7/8 of enc DMA changes nothing).
  - enc is pre-transposed on the host to (E, T) per batch so the PE
    contraction tiles (E on partitions) load with contiguous free dims.
  - ht/ctx/acc/scratch are fp16 (DVE 16-bit 2x mode); DVE scalar operands
    (exps columns) must be f32, so exps stays f32r with bitcasts.
  - Matmul order is k-outer (PSUM banks psA/psB alternate per MM).
  - Pooling of batch i runs on DVE (serial scalar_tensor_tensor chain)
    interleaved into batch i+1's matmul stream; the final ones x acc
    partition-reduce on PE is deferred REDUCE_DELAY chains so the PE
    stream never waits on the DVE chain. The last batch splits pooling
    PE/DVE half-half (combined in one PSUM accumulation group) to halve
    the exposed tail.
  - Softmax uses DVE free-dim reduce + GPSIMD partition_all_reduce; the
    1/Z normalization is folded into the host-side divide by zout.
  - The dyn-loop bench variant uses branch-prefetch hints + staggered
    semaphore reset at the For_i back-edge (no all-engine barrier).
  - Steady-state HW time ~235 us/core quiet, ~290-330 us under heavy
    machine load (PE throttle + HBM contention from co-tenants).
"""
import contextlib

import numpy as np
import ml_dtypes

import concourse.bacc as bacc
import concourse.bass_isa as bass_isa
import concourse.mybir as mybir
from concourse.bass_utils import run_bass_kernel_spmd
from concourse.tile import TileContext, add_dep_helper

B, T, E = 32, 2048, 1024
NCORES = 8
BPC = B // NCORES          # batches per core
NT = T // 128              # 16 t-tiles per batch
NK = E // 128              # 8 k-tiles (contraction)
NT512 = T // 512           # 4 groups of 4 t-tiles
POOL_DELAY = 2             # m-chains of next batch before prev pooling
REDUCE_DELAY = 12          # m-chains before prev batch's PE pool-reduce

f32 = mybir.dt.float32
f32r = mybir.dt.float32r
f16 = mybir.dt.float16
bf16 = mybir.dt.bfloat16
AF = mybir.ActivationFunctionType
ALU = mybir.AluOpType
AX = mybir.AxisListType


def _build(with_bias, repeat=1, dyn_loop=False, ablate=""):
    ab = set(ablate.split(",")) if ablate else set()
    pool_delay = POOL_DELAY
    psum_bufs, et_bufs = 3, 2
    for tok in list(ab):
        if tok.startswith("pd"):
            pool_delay = int(tok[2:]); ab.discard(tok)
        elif tok.startswith("psum"):
            psum_bufs = int(tok[4:]); ab.discard(tok)
        elif tok.startswith("et"):
            et_bufs = int(tok[2:]); ab.discard(tok)
    dve_pool = "nodvepool" not in ab
    ab.discard("dvepool"); ab.discard("nodvepool")
    psplit = "nopsplit" not in ab
    ab.discard("psplit"); ab.discard("nopsplit")
    # k-outer (PSUM-bank-alternating) matmul order is the default: psA/psB
    # drains overlap the other bank's fills. "nouter" reverts to n-outer.
    kouter = "nouter" not in ab
    ab.discard("kouter"); ab.discard("nouter")
    # dyn-loop back-edge: branch-prefetch hints (body >> one IRAM block) and
    # staggered semaphore reset (no all-engine barrier) are on by default
    loop_hint = "nohint" not in ab
    ab.discard("hint"); ab.discard("nohint")
    loop_stag = "nostag" not in ab
    ab.discard("stag"); ab.discard("nostag")
    nc = bacc.Bacc(None)
    enc = nc.declare_dram_parameter("enc", [BPC, E, T], f16, isOutput=False)
    if dyn_loop:
        nrep = nc.declare_dram_parameter("nrep", [1, 1], mybir.dt.int32,
                                         isOutput=False)
    ctxv = nc.declare_dram_parameter("ctx", [BPC, E], f16, isOutput=False)
    W = nc.declare_dram_parameter("W", [E, E], f16, isOutput=False)
    bvec = nc.declare_dram_parameter("b", [2, E], f32, isOutput=False)
    out = nc.declare_dram_parameter("out", [BPC, E], f32, isOutput=True)
    zout = nc.declare_dram_parameter("zout", [BPC, 128], f32, isOutput=True)

    with TileContext(nc) as tc:
        with (
            tc.tile_pool(name="const", bufs=1) as cpool,
            tc.tile_pool(name="ht2", bufs=2) as htpool2,
            tc.tile_pool(name="ht1", bufs=1) as htpool1,
            tc.tile_pool(name="et", bufs=et_bufs) as etpool,
            tc.tile_pool(name="work", bufs=2) as wpool,
            tc.tile_pool(name="psum", bufs=psum_bufs, space="PSUM") as psum_pool,
            tc.tile_pool(name="ppool", bufs=1, space="PSUM") as ppool,
        ):
            # --- constants ---
            # In the single-shot program, W tile loads are interleaved with
            # the first batch's enc tile loads so the first matmul chain
            # starts after ~0.5MB of DMA instead of ~6MB.
            w_t = []
            for k in range(NK):
                wt = cpool.tile([128, E], f16, tag=f"w{k}", name=f"w_t{k}")
                if dyn_loop:
                    nc.sync.dma_start(out=wt[:], in_=W[k * 128:(k + 1) * 128, :])
                w_t.append(wt)
            w_loaded = dyn_loop
            if with_bias:
                b_f = cpool.tile([2, E], f32)
                nc.sync.dma_start(out=b_f[:], in_=bvec[:])
                b_t = cpool.tile([2, E], bf16)
                nc.vector.tensor_copy(b_t[:], b_f[:])
                zero_s = cpool.tile([2, 128], f32)
                nc.vector.memset(zero_s[:], 0.0)
                ones_b = cpool.tile([2, 128], bf16)
                nc.scalar.activation(ones_b[:], zero_s[:], AF.Copy,
                                     bias=1.0, scale=0.0)

            if dve_pool:
                zero_o = cpool.tile([128, 1], f32)
                nc.vector.memset(zero_o[:], 0.0)
                ones_r = cpool.tile([128, 1], f16)
                nc.scalar.activation(ones_r[:], zero_o[:], AF.Copy,
                                     bias=1.0, scale=0.0)

            # per-batch state carried between emission phases
            state = {}

            def pe(bi):
                return bi

            def mm_pair(dst, lhsT, rhs, start, stop):
                return nc.tensor.matmul(dst, lhsT, rhs, start=start,
                                        stop=stop)
            loop_cm = contextlib.nullcontext()
            if dyn_loop:
                nrep_t = cpool.tile([1, 1], mybir.dt.int32)
                nc.sync.dma_start(out=nrep_t[:], in_=nrep[:])
                nval = nc.values_load(nrep_t[0:1, 0:1])
                hints = (tuple(mybir.EngineType) if loop_hint else ())
                loop_cm = tc.For_i(0, nval, 1, hint_engines=hints,
                                   staggered_reset=loop_stag)

            pending_reduce = {}
            # DVE pooling ops are drained 2-per-chain from this queue so the
            # ~9us burst of acc-chain SBUF traffic (ht + acc reads, acc
            # writes) spreads across the batch instead of contending with
            # the PE operand streams all at once.
            acc_queue = []

            def _emit_acc_chains(i, exps, ht_b, t0, t1):
                # sum_{t in [t0,t1)} ht[t] * exps[:, t] via two parity
                # chains, each ping-ponging between two tiles; returns the
                # combined [128, E] f16 tile.
                acc = [[wpool.tile([128, E], f16, tag=f"acc{j}{s}",
                                   name=f"acc_{i}_{j}{s}") for s in range(2)]
                       for j in range(2)]
                steps = {0: 0, 1: 0}
                lastt = {}
                ts = list(range(t0, t1))
                for idx, t in enumerate(ts):
                    j = idx % 2
                    s = steps[j]
                    if s == 0:
                        nc.vector.tensor_scalar_mul(
                            acc[j][0][:], ht_b[t][:],
                            exps[:, t:t + 1].bitcast(f32))
                    else:
                        nc.vector.scalar_tensor_tensor(
                            out=acc[j][s % 2][:],
                            in0=ht_b[t][:],
                            scalar=exps[:, t:t + 1].bitcast(f32),
                            in1=acc[j][(s + 1) % 2][:],
                            op0=ALU.mult, op1=ALU.add)
                    lastt[j] = s % 2
                    steps[j] += 1
                if steps[1] == 0:
                    return acc[0][lastt[0]]
                comb = wpool.tile([128, E], f16, tag="acccomb",
                                  name=f"acccomb_{i}")
                nc.vector.tensor_add(comb[:], acc[0][lastt[0]][:],
                                     acc[1][lastt[1]][:])
                return comb

            def emit_pooling(i):
                if "pool" in ab:
                    return
                exps, ht_b, rz, b = state[i]
                # last batch: PE pooling (PE is idle at the tail and its
                # 32-matmul chain is ~10us shorter than the serial DVE chain)
                last_i = repeat * BPC - 1
                if dve_pool and i != last_i:
                    # acc = sum_t ht[t] * exps[:, t] on DVE as two
                    # independent even/odd-parity chains (interleaved on the
                    # DVE FIFO, so dependent-op drain bubbles are hidden and
                    # the serial depth halves); ops are queued as thunks and
                    # drained 2 per m-chain. The final PE partition-reduce
                    # is deferred to emit_pool_reduce so the PE stream never
                    # waits on the chains.
                    acc = [[wpool.tile([128, E], f16, tag=f"acc{j}{s}",
                                       name=f"acc_{i}_{j}{s}")
                            for s in range(2)] for j in range(2)]
                    steps = [0, 0]
                    for idx in range(NT):
                        t, j = idx, idx % 2
                        s = steps[j]
                        if s == 0:
                            acc_queue.append(
                                lambda t=t, j=j: nc.vector.tensor_scalar_mul(
                                    acc[j][0][:], ht_b[t][:],
                                    exps[:, t:t + 1].bitcast(f32)))
                        else:
                            acc_queue.append(
                                lambda t=t, j=j, s=s:
                                nc.vector.scalar_tensor_tensor(
                                    out=acc[j][s % 2][:],
                                    in0=ht_b[t][:],
                                    scalar=exps[:, t:t + 1].bitcast(f32),
                                    in1=acc[j][(s + 1) % 2][:],
                                    op0=ALU.mult, op1=ALU.add))
                        steps[j] += 1
                    lastE = (steps[0] - 1) % 2
                    lastO = (steps[1] - 1) % 2

                    def comb_thunk(i=i, b=b, acc=acc, lastE=lastE,
                                   lastO=lastO):
                        comb = wpool.tile([128, E], f16, tag="acccomb",
                                          name=f"acccomb_{i}")
                        nc.vector.tensor_add(comb[:], acc[0][lastE][:],
                                             acc[1][lastO][:])
                        pending_reduce[i] = (comb, b)

                    acc_queue.append(comb_thunk)
                else:
                    # tail batch: split pooling across PE (t < SPLIT, direct
                    # exps16 x ht matmuls) and DVE (t >= SPLIT, serial acc
                    # chain) so the exposed tail is ~halved; the DVE partial
                    # joins the same PSUM accumulation group via a final
                    # ones x acc matmul with start=False.
                    SPLIT = NT // 2
                    exps16 = wpool.tile([128, NT], f16, tag="exps16",
                                        name=f"exps16_{i}")
                    nc.vector.tensor_copy(exps16[:], exps[:].bitcast(f32))
                    last = _emit_acc_chains(i, exps, ht_b, SPLIT, NT)
                    ps_o = ppool.tile([1, E], f32, tag="ps_o", name=f"ps_o{i}")
                    for n in range(2):
                        sl = slice(n * 512, (n + 1) * 512)
                        for t in range(SPLIT):
                            pe(nc.tensor.matmul(ps_o[:, sl],
                                                exps16[:, t:t + 1],
                                                ht_b[t][:, sl],
                                                start=(t == 0),
                                                stop=False))
                    for n in range(2):
                        sl = slice(n * 512, (n + 1) * 512)
                        pe(nc.tensor.matmul(ps_o[:, sl], ones_r[:],
                                            last[:, sl],
                                            start=False, stop=(n == 1)))
                    _emit_out(i, ps_o, state[i][3])

            def emit_pool_reduce(i):
                if i not in pending_reduce:
                    return
                last, b = pending_reduce.pop(i)
                ps_o = ppool.tile([1, E], f32, tag="ps_o", name=f"ps_o{i}")
                for n in range(2):
                    sl = slice(n * 512, (n + 1) * 512)
                    pe(nc.tensor.matmul(ps_o[:, sl], ones_r[:], last[:, sl],
                                        start=True, stop=True))
                _emit_out(i, ps_o, b)

            def _emit_out(i, ps_o, b):
                out_sb = wpool.tile([1, E], f32, tag="out_sb", name=f"out_sb{i}")
                nc.scalar.activation(out_sb[:], ps_o[:], AF.Copy)
                nc.sync.dma_start(out=out[b:b + 1, :], in_=out_sb[:])

            with loop_cm:
                for i in range(repeat * BPC):
                    b = i % BPC
                    ctx_b = wpool.tile([128, E], f16, tag="ctx_b", name=f"ctx_b{i}")
                    nc.sync.dma_start(out=ctx_b[:],
                                      in_=ctxv[b:b + 1, :].to_broadcast((128, E)))
                    scores = wpool.tile([128, NT], f32, tag="scores",
                                        name=f"scores{i}")
                    # tiles written before prev batch's pooling is emitted need
                    # double buffering; later ones can reuse a single slot
                    ht = [(htpool2 if t < pool_delay + 2 else htpool1).tile(
                              [128, E], f16, tag=f"ht{t}", name=f"ht_{i}_{t}")
                          for t in range(NT)]

                    chain_idx = 0
                    et_tiles = None
                    for t512 in range(NT512):
                        first_group = not w_loaded and psplit and not ab
                        et_tiles = []
                        for k in range(NK):
                            if "dma" in ab and k > 0:
                                et_tiles.append(et_tiles[0])
                                continue
                            et = etpool.tile([128, 512], f16, tag=f"et{k}",
                                             name=f"et_{i}_{t512}_{k}")
                            src = enc[b, k * 128:(k + 1) * 128,
                                      t512 * 512:(t512 + 1) * 512]
                            if not w_loaded:
                                if first_group:
                                    # n=0 half of W first: the first 4 chains
                                    # only need cols 0:512, so the first
                                    # matmuls start after ~4MB of DMA not 6MB
                                    nc.sync.dma_start(
                                        out=w_t[k][:, 0:512],
                                        in_=W[k * 128:(k + 1) * 128, 0:512])
                                else:
                                    nc.sync.dma_start(
                                        out=w_t[k][:],
                                        in_=W[k * 128:(k + 1) * 128, :])
                            if "dma" not in ab or k == 0:
                                nc.sync.dma_start(out=et[:], in_=src)
                            et_tiles.append(et)
                        if first_group:
                            for k in range(NK):
                                nc.sync.dma_start(
                                    out=w_t[k][:, 512:1024],
                                    in_=W[k * 128:(k + 1) * 128, 512:1024])
                        w_loaded = True
                        if first_group:
                            # n-outer over the whole group: all four m-chains
                            # run on the n=0 W halves before any n=1 chain
                            ps_h = {}
                            for nn in range(2):
                                nsl = slice(nn * 512, (nn + 1) * 512)
                                for m in range(4):
                                    t = t512 * 4 + m
                                    msl = slice(m * 128, (m + 1) * 128)
                                    tag = "psA" if nn == 0 else "psB"
                                    ph = psum_pool.tile(
                                        [128, 512], f32, tag=tag,
                                        name=f"ps{tag[-1]}_{i}_{t}")
                                    ps_h[(m, nn)] = ph
                                    for k in range(NK):
                                        mm_pair(
                                            ph[:], et_tiles[k][:, msl],
                                            w_t[k][:, nsl], start=(k == 0),
                                            stop=(k == NK - 1
                                                  and not with_bias))
                                    if with_bias:
                                        pe(nc.tensor.matmul(
                                            ph[:], ones_b[:], b_t[:, nsl],
                                            start=False, stop=True))
                                    nc.scalar.activation(ht[t][:, nsl],
                                                         ph[:], AF.Tanh)
                                    if nn == 1:
                                        scratch = wpool.tile(
                                            [128, E], f16, tag="scratch",
                                            name=f"scr_{i}_{t}")
                                        nc.vector.scalar_tensor_tensor(
                                            out=scratch[:],
                                            in0=ht[t][:],
                                            scalar=1.0, in1=ctx_b[:],
                                            op0=ALU.mult, op1=ALU.mult,
                                            accum_out=scores[:, t:t + 1])
                                        chain_idx += 1
                            continue
                        for m in range(4):
                            t = t512 * 4 + m
                            msl = slice(m * 128, (m + 1) * 128)
                            if psplit:
                                psA = psum_pool.tile([128, 512], f32, tag="psA",
                                                     name=f"psA_{i}_{t}")
                                psB = psum_pool.tile([128, 512], f32, tag="psB",
                                                     name=f"psB_{i}_{t}")
                                ps_halves = [psA, psB]
                            else:
                                ps = psum_pool.tile([128, E], f32, tag="ps",
                                                    name=f"ps_{i}_{t}")
                            nk_eff = 1 if "mm" in ab else NK
                            korder = kouter
                            if korder:
                                seq = [(k, n) for k in range(nk_eff)
                                       for n in range(2)]
                            else:
                                seq = [(k, n) for n in range(2)
                                       for k in range(nk_eff)]
                            for k, n in seq:
                                nsl = slice(n * 512, (n + 1) * 512)
                                dst = (ps_halves[n][:] if psplit
                                       else ps[:, nsl])
                                mm_pair(
                                    dst, et_tiles[k][:, msl],
                                    w_t[k][:, nsl], start=(k == 0),
                                    stop=(k == nk_eff - 1 and not with_bias))
                            if with_bias:
                                for n in range(2):
                                    nsl = slice(n * 512, (n + 1) * 512)
                                    dst = (ps_halves[n][:] if psplit
                                           else ps[:, nsl])
                                    pe(nc.tensor.matmul(dst, ones_b[:],
                                                        b_t[:, nsl],
                                                        start=False,
                                                        stop=True))
                            if psplit:
                                nc.scalar.activation(ht[t][:, 0:512],
                                                     psA[:], AF.Tanh)
                                nc.scalar.activation(ht[t][:, 512:1024],
                                                     psB[:], AF.Tanh)
                            else:
                                nc.scalar.activation(ht[t][:], ps[:], AF.Tanh)
                            scratch = wpool.tile([128, E], f16, tag="scratch",
                                                 name=f"scr_{i}_{t}")
                            if "stt" not in ab:
                                nc.vector.scalar_tensor_tensor(
                                    out=scratch[:], in0=ht[t][:],
                                    scalar=1.0, in1=ctx_b[:], op0=ALU.mult,
                                    op1=ALU.mult, accum_out=scores[:, t:t + 1])
                            elif t == 0:
                                nc.vector.memset(scores[:], 0.5)
                            chain_idx += 1
                            if i > 0 and chain_idx == pool_delay:
                                emit_pooling(i - 1)
                            for _ in range(2):
                                if acc_queue:
                                    acc_queue.pop(0)()
                            if i > 0 and chain_idx == REDUCE_DELAY:
                                emit_pool_reduce(i - 1)

                    # safety: drain any pooling thunks not yet emitted
                    while acc_queue:
                        acc_queue.pop(0)()

                    # softmax for batch b
                    if "softmax" in ab:
                        exps = wpool.tile([128, NT], f32r, tag="exps",
                                          name=f"exps{i}")
                        nc.vector.memset(exps[:].bitcast(f32), 0.5)
                        state[i] = (exps, ht, None, b)
                        continue
                    rmax = wpool.tile([128, 1], f32, tag="rmax", name=f"rmax{i}")
                    nc.vector.tensor_reduce(rmax[:], scores[:], axis=AX.X,
                                            op=ALU.max)
                    m128 = wpool.tile([128, 1], f32, tag="m128", name=f"m128{i}")
                    nc.gpsimd.partition_all_reduce(
                        m128[:], rmax[:], channels=128,
                        reduce_op=bass_isa.ReduceOp.max)
                    negm = wpool.tile([128, 1], f32, tag="negm", name=f"negm{i}")
                    nc.scalar.activation(negm[:], m128[:], AF.Copy, scale=-1.0)
                    exps = wpool.tile([128, NT], f32r, tag="exps", name=f"exps{i}")
                    zrow = wpool.tile([128, 1], f32, tag="zrow", name=f"zrow{i}")
                    nc.scalar.activation(exps[:], scores[:], AF.Exp, bias=negm[:],
                                         accum_out=zrow[:])
                    nc.sync.dma_start(out=zout[b:b + 1, :], in_=zrow[:])
                    state[i] = (exps, ht, None, b)

                emit_pooling(repeat * BPC - 1)
            state.clear()
    nc.finalize()
    return nc


_cache = {}


def _get_nc(with_bias, repeat=1, dyn_loop=False, ablate=""):
    key = (with_bias, repeat, dyn_loop, ablate)
    if key not in _cache:
        _cache[key] = _build(with_bias, repeat, dyn_loop, ablate)
    return _cache[key]


def _run(enc, ctx, W, b, trace=False, tmpdir=None):
    enc = np.asarray(enc, dtype=np.float32)
    ctx = np.ascontiguousarray(np.asarray(ctx, dtype=np.float32))
    W = np.ascontiguousarray(np.asarray(W, dtype=np.float32))
    b = np.asarray(b, dtype=np.float32).reshape(1, E)

    with_bias = bool(np.any(b))
    b_hi = b.astype(ml_dtypes.bfloat16).astype(np.float32)
    b_lo = (b - b_hi).astype(ml_dtypes.bfloat16).astype(np.float32)
    b2 = np.concatenate([b_hi, b_lo], axis=0)

    nc = _get_nc(with_bias)
    enc16 = enc.astype(np.float16)
    W16 = W.astype(np.float16)
    ctx16 = ctx.astype(np.float16)
    in_maps = [
        {"enc": np.ascontiguousarray(
             enc16[c * BPC:(c + 1) * BPC].transpose(0, 2, 1)),
         "ctx": ctx16[c * BPC:(c + 1) * BPC],
         "W": W16, "b": b2}
        for c in range(NCORES)
    ]
    res = run_bass_kernel_spmd(nc, in_maps, list(range(NCORES)),
                               trace=trace, tmpdir=tmpdir)
    outp = np.concatenate([res.results[c]["out"] for c in range(NCORES)],
                          axis=0).astype(np.float32)
    zsum = np.concatenate([res.results[c]["zout"] for c in range(NCORES)],
                          axis=0).astype(np.float64).sum(axis=1)
    outp = (outp / zsum[:, None]).astype(np.float32)
    return outp, res


def kernel(enc, ctx, W, b):
    outp, _ = _run(enc, ctx, W, b)
    return outp

